# revision 23
# baseline (speedup 1.0000x reference)
"""Trainium2 Bass kernel for FerroelectricBasisConv2d.

Math (derived from the reference):
  dx = x - stop_gradient(x) = 0  =>  is_up = sigmoid(0) = 0.5 exactly.
  target_sign = 1 - sigmoid(10*(-x - Ec)) = sigmoid(10*(x + Ec))
  branch_momentum = 0.8 + 0.2*sigmoid(10*(x+Ec)),  shifted = x + Ec*bm
  out[co, f] = const[co] + sum_r w[co,r] * tanh(k*x + 0.8*k*Ec + 0.2*k*Ec*s)
  with r = (ci, nb, kh, kw) (432 terms), w = coef*Ps,
  const[co] = sum_r coef*bias + out_bias[co], s = sigmoid(10*x + 10*Ec).

Device layout: r on partitions (3 full 128-row chunks + one 48-row tail),
spatial f = (b, ho, wo) = 4096 on the free axis.  Cout=32 sharded 4 per core
across 8 cores.  Per iteration:
  ScalarE  s = sigmoid(10*x + b10)          (scale/bias fused into the act)
  VectorE  m = x*k + c1                     (tensor_scalar, 2x fp32 mode)
  VectorE  m = s*c2 + m                     (scalar_tensor_tensor, in place)
  ScalarE  v = tanh(m)
  TensorE  psum[32j] += w . v               (1-col lhsT, col-group j)
The channel-pair tail iterations share one x tile (rows 0:48 / 48:96) and
fold the per-channel constant via saturated-tanh ones-rows (rows 96/97).
PSUM rows 0/32/64/96 are copied to SBUF and DMAd out per channel.
"""

import numpy as np
from contextlib import ExitStack

import ml_dtypes

import concourse.bass as bass
import concourse.tile as tile
from concourse import bacc, mybir
from concourse.bass_utils import run_bass_kernel_spmd

# Problem shapes (hardcoded per contract).
B, Cin, H, W = 4, 16, 32, 32
Cout, NB, KH, KW = 32, 3, 3, 3
R = Cin * NB * KH * KW        # 432
F = B * H * W                 # 4096
NCORES = 8
CO_PER_CORE = Cout // NCORES  # 4
NFULL = R // 128              # 3 full 128-row chunks
TAIL = R - NFULL * 128        # 48
NITER = NFULL * CO_PER_CORE + 2

ALPHA = 0.8
GATE = 10.0
MM_SEG = 512  # fp32 moving-operand / PSUM-bank limit


def _iter_specs():
    """Iteration table, j-major with channel-pair tails early so the
    PSUM->SBUF row copies overlap remaining compute.

    Each entry: dict(x=tile idx, base=psum row, ncols=lhsT cols, start, stop,
    tpos=tile_position, rows=[(plo, phi, co_idx, rlo, rhi, wt_col)],
    const=[(partition, co_idx, wt_col)], fin=[channels finalized])."""
    def full(c, j):
        return dict(x=c, base=32 * j, ncols=1, start=(c == 0), stop=(c == 2),
                    tpos=(0, 32 * j), rows=[(0, 128, j, c * 128, (c + 1) * 128, 0)],
                    const=[], fin=([j] if c == 2 else []))

    def tailp(jA, jB):
        # const rows: hi/lo split so a bf16 weight tensor still carries the
        # channel constant to ~fp32 accuracy (two saturated-tanh ones-rows)
        return dict(x=3, base=32 * jA, ncols=64, start=False, stop=False,
                    tpos=(0, 32 * jA),
                    rows=[(0, TAIL, jA, NFULL * 128, R, 0),
                          (TAIL, 2 * TAIL, jB, NFULL * 128, R, 32)],
                    const=[(96, jA, 0, "hi"), (98, jA, 0, "lo"),
                           (97, jB, 32, "hi"), (99, jB, 32, "lo")],
                    fin=[])

    # per channel the accumulation order is c0 (start), tail, c1, c2
    # (stop+fin), so every channel finalizes on a full-chunk iteration and
    # the tail iterations sit mid-stream.
    specs = []
    specs.append(full(0, 0))
    specs.append(full(0, 1))
    specs.append(tailp(0, 1))
    specs.append(full(1, 0))
    specs.append(full(1, 1))
    specs.append(full(2, 0))
    specs.append(full(2, 1))
    specs.append(full(0, 2))
    specs.append(full(0, 3))
    specs.append(tailp(2, 3))
    specs.append(full(1, 2))
    specs.append(full(1, 3))
    specs.append(full(2, 2))
    specs.append(full(2, 3))
    return specs


def _build_bass(mm_dtype=mybir.dt.float32):
    nc = bacc.Bacc(
        "TRN2",
        target_bir_lowering=False,
        debug=False,
        enable_asserts=False,
        num_devices=NCORES,
    )
    f32 = mybir.dt.float32
    xx = nc.dram_tensor("xx", [4, 128, F], f32, kind="ExternalInput")
    par = nc.dram_tensor("par", [128, NITER, 4], f32, kind="ExternalInput")
    wt = nc.dram_tensor("wt", [128, NITER, 64], mm_dtype, kind="ExternalInput")
    out = nc.dram_tensor("out", [4, F], f32, kind="ExternalOutput")

    with ExitStack() as ctx:
        tc = ctx.enter_context(tile.TileContext(nc))
        singles = ctx.enter_context(tc.tile_pool(name="singles", bufs=1))
        xpool = ctx.enter_context(tc.tile_pool(name="xpool", bufs=1))
        spool = ctx.enter_context(tc.tile_pool(name="spool", bufs=3))
        tpool = ctx.enter_context(tc.tile_pool(name="tpool", bufs=3))
        vpool = ctx.enter_context(tc.tile_pool(name="vpool", bufs=3))
        psum_pool = ctx.enter_context(tc.tile_pool(name="psum", bufs=1, space="PSUM"))

        # Warm the activation table set (sigmoid_and_others, includes tanh)
        # before any DMA completes, so the ~2.7us load is off the critical path.
        zt = singles.tile([1, 1], f32, tag="zt")
        nc.vector.memset(zt[:], 0.0)
        nc.scalar.activation(zt[:], zt[:], mybir.ActivationFunctionType.Sigmoid)

        # DMA order follows first-use: params, x0 (in quarters so the first
        # iteration can start on the first quarter), tail tile x3, x1, wt, x2.
        par_sb = singles.tile([128, NITER, 4], f32, tag="par")
        nc.sync.dma_start(par_sb[:], par[:, :, :])
        xts = []
        for i in range(4):
            xt = xpool.tile([128, F], f32, tag=f"x{i}")
            xts.append(xt)
        for q in range(4):
            nc.sync.dma_start(xts[0][:, q * 1024:(q + 1) * 1024],
                              xx[0, :, q * 1024:(q + 1) * 1024])
        nc.sync.dma_start(xts[3][:], xx[3, :, :])
        nc.sync.dma_start(xts[1][:], xx[1, :, :])
        wt_sb = singles.tile([128, NITER, 64], mm_dtype, tag="wt")
        nc.sync.dma_start(wt_sb[:], wt[:, :, :])
        nc.sync.dma_start(xts[2][:], xx[2, :, :])

        psum_t = psum_pool.tile([128, F], f32, tag="acc")
        out_sb = singles.tile([128, F], f32, tag="osb")

        Act = mybir.ActivationFunctionType
        Op = mybir.AluOpType
        specs = _iter_specs()
        nspec = len(specs)
        for i, sp in enumerate(specs):
            xt = xts[sp["x"]]
            # first iteration in quarters (overlaps the piecewise x0 DMA),
            # last iteration in halves (pipelines the kernel drain)
            npiece = 4 if i == 0 else (2 if i == nspec - 1 else 1)
            fp = F // npiece
            for q in range(npiece):
                flo, fhi = q * fp, (q + 1) * fp
                s_t = spool.tile([128, fp], f32, tag="s")
                nc.scalar.activation(s_t[:], xt[:, flo:fhi], Act.Sigmoid,
                                     bias=par_sb[:, i, 0:1], scale=GATE)
                # t = s*(0.2*Ec) + x; the k multiply and the 0.8*k*Ec add are
                # folded into the tanh activation's per-partition scale/bias
                t_t = tpool.tile([128, fp], f32, tag="t")
                nc.vector.scalar_tensor_tensor(t_t[:], s_t[:],
                                               par_sb[:, i, 3:4],
                                               xt[:, flo:fhi],
                                               Op.mult, Op.add)
                v_t = vpool.tile([128, fp], mm_dtype, tag="v")
                nc.scalar.activation(v_t[:], t_t[:], Act.Tanh,
                                     bias=par_sb[:, i, 2:3],
                                     scale=par_sb[:, i, 1:2])
                nb, nco = sp["base"], sp["ncols"]
                for seg in range(fp // MM_SEG):
                    nc.tensor.matmul(
                        psum_t[nb:nb + nco,
                               flo + seg * MM_SEG:flo + (seg + 1) * MM_SEG],
                        wt_sb[:, i, 0:nco],
                        v_t[:, seg * MM_SEG:(seg + 1) * MM_SEG],
                        start=sp["start"], stop=sp["stop"],
                        tile_position=sp["tpos"],
                    )
                for j in sp["fin"]:
                    src = psum_t[32 * j:32 * j + 1, flo:fhi]
                    dst = out_sb[32 * j:32 * j + 1, flo:fhi]
                    if i == nspec - 1:
                        nc.scalar.copy(dst, src)  # ScalarE free at the tail
                    else:
                        nc.vector.tensor_copy(dst, src)
                    nc.sync.dma_start(out[j:j + 1, flo:fhi], dst)

    nc.compile()
    return nc


def _host_prep(x, k, Ec, Ps, bias, coef, out_bias, w_np_dtype):
    """Build the unfolded X tiles (core-independent) and per-core params."""
    f32 = np.float32
    x = np.asarray(x, f32)
    xp = np.pad(x, ((0, 0), (0, 0), (1, 1), (1, 1)))
    # X[r, f]: r = (ci, nb, kh, kw), f = (b, ho, wo)
    Xf = np.empty((Cin, NB, KH, KW, F), f32)
    for kh in range(KH):
        for kw in range(KW):
            win = xp[:, :, kh:kh + H, kw:kw + W]              # [B, Cin, 32, 32]
            win = win.transpose(1, 0, 2, 3).reshape(Cin, F)   # [Cin, F]
            Xf[:, :, kh, kw, :] = win[:, None, :]
    X432 = Xf.reshape(R, F)

    xx = np.zeros((4, 128, F), f32)
    xx[0:NFULL] = X432[0:NFULL * 128].reshape(NFULL, 128, F)
    xx[3, 0:TAIL] = X432[NFULL * 128:]
    xx[3, TAIL:2 * TAIL] = X432[NFULL * 128:]

    k2 = np.asarray(k, f32).reshape(Cout, R)
    Ec2 = np.asarray(Ec, f32).reshape(Cout, R)
    Ps2 = np.asarray(Ps, f32).reshape(Cout, R)
    bias2 = np.asarray(bias, f32).reshape(Cout, R)
    coef2 = np.asarray(coef, f32).reshape(Cout, R)
    ob = np.asarray(out_bias, f32).reshape(Cout)

    b10 = GATE * Ec2
    c1 = ALPHA * k2 * Ec2          # tanh bias
    c2k = (1.0 - ALPHA) * Ec2      # STT scalar (k folded into tanh scale)
    w = coef2 * Ps2
    const = (coef2 * bias2).sum(axis=1) + ob

    specs = _iter_specs()
    in_maps = []
    for d in range(NCORES):
        cos = [d * CO_PER_CORE + jj for jj in range(CO_PER_CORE)]
        PAR = np.zeros((128, NITER, 4), f32)
        WT = np.zeros((128, NITER, 64), f32)
        for i, sp in enumerate(specs):
            for (plo, phi, j, rlo, rhi, col) in sp["rows"]:
                co = cos[j]
                PAR[plo:phi, i, 0] = b10[co, rlo:rhi]
                PAR[plo:phi, i, 1] = k2[co, rlo:rhi]
                PAR[plo:phi, i, 2] = c1[co, rlo:rhi]
                PAR[plo:phi, i, 3] = c2k[co, rlo:rhi]
                WT[plo:phi, i, col] = w[co, rlo:rhi]
            for (p, j, col, part) in sp["const"]:
                # arg = 25 -> tanh = 1.0 exactly; weight = channel constant
                PAR[p, i, 2] = 25.0
                hi = w_np_dtype(np.float32(const[cos[j]]))
                if part == "hi":
                    WT[p, i, col] = np.float32(hi)
                else:
                    WT[p, i, col] = np.float32(const[cos[j]]) - np.float32(hi)
        in_maps.append({
            "xx": xx,
            "par": PAR,
            "wt": WT.astype(w_np_dtype),
        })
    return in_maps


_nc_cache = {}
last_results = None  # BassKernelResults from the most recent run

_MM_MODES = {
    "fp32": (mybir.dt.float32, np.float32),
    "bf16": (mybir.dt.bfloat16, ml_dtypes.bfloat16),
}
MM_MODE = "bf16"


def _get_nc():
    key = MM_MODE
    if key not in _nc_cache:
        _nc_cache[key] = _build_bass(mm_dtype=_MM_MODES[key][0])
    return _nc_cache[key]


def kernel(x, k, Ec, Ps, bias, coef, out_bias, _trace=False):
    global last_results
    in_maps = _host_prep(x, k, Ec, Ps, bias, coef, out_bias, _MM_MODES[MM_MODE][1])
    try:
        res = run_bass_kernel_spmd(_get_nc(), in_maps,
                                   core_ids=list(range(NCORES)), trace=_trace)
    except ModuleNotFoundError:
        # axon NTFF profiling hook unavailable -> run without trace
        res = run_bass_kernel_spmd(_get_nc(), in_maps,
                                   core_ids=list(range(NCORES)), trace=False)
    last_results = res
    o = np.concatenate([r["out"] for r in res.results], axis=0)  # [32, F]
    o = o.reshape(Cout, B, H, W).transpose(1, 0, 2, 3)
    return np.ascontiguousarray(o.astype(np.float32))


# revision 35
# speedup vs baseline: 7643.5174x; 7643.5174x over previous
"""Trainium2 Bass kernel for FerroelectricBasisConv2d.

Math (derived from the reference):
  dx = x - stop_gradient(x) = 0  =>  is_up = sigmoid(0) = 0.5 exactly.
  target_sign = 1 - sigmoid(10*(-x - Ec)) = sigmoid(10*(x + Ec))
  branch_momentum = 0.8 + 0.2*sigmoid(10*(x+Ec)),  shifted = x + Ec*bm
  out[co, f] = const[co] + sum_r w[co,r] * tanh(k*x + 0.8*k*Ec + 0.2*k*Ec*s)
  with r = (ci, nb, kh, kw) (432 terms), w = coef*Ps,
  const[co] = sum_r coef*bias + out_bias[co], s = sigmoid(10*x + 10*Ec).

Device layout: r on partitions (3 full 128-row chunks + one 48-row tail),
spatial f = (b, ho, wo) = 4096 on the free axis.  Cout=32 sharded 4 per core
across 8 cores.  Per iteration (14 per core, the engine-count minimum):
  ScalarE  s = sigmoid(10*x + b10)        (scale=10, bias=10*Ec fused)
  VectorE  t = s*(0.2*Ec) + x             (one scalar_tensor_tensor)
  ScalarE  v = tanh(k*t + 0.8*k*Ec)       (per-partition scale/bias fused)
  TensorE  psum[32j] += w . v             (fp16 1-col lhsT, col-group j)
ScalarE is the bound: 28 activations/core is the floor for 2 transcendentals
over 14 row-chunks.  The channel-pair tail iterations share one x tile (rows
0:48 / 48:96) and fold the per-channel constant via saturated-tanh ones-rows
(96-99, hi/lo split).  PSUM rows 0/32/64/96 are copied to SBUF (DVE mid-
stream, ScalarE at the drain) and DMAd out per channel.  First iteration runs
in quarters against a piecewise x DMA; the last in quarters to pipeline the
drain.
"""

import numpy as np
from contextlib import ExitStack

import ml_dtypes

import concourse.bass as bass
import concourse.tile as tile
from concourse import bacc, mybir
from concourse.bass_utils import run_bass_kernel_spmd

# Problem shapes (hardcoded per contract).
B, Cin, H, W = 4, 16, 32, 32
Cout, NB, KH, KW = 32, 3, 3, 3
R = Cin * NB * KH * KW        # 432
F = B * H * W                 # 4096
NCORES = 8
CO_PER_CORE = Cout // NCORES  # 4
NFULL = R // 128              # 3 full 128-row chunks
TAIL = R - NFULL * 128        # 48
NITER = NFULL * CO_PER_CORE + 2

ALPHA = 0.8
GATE = 10.0
MM_SEG = 512  # fp32 moving-operand / PSUM-bank limit


def _iter_specs():
    """Iteration table, j-major with channel-pair tails early so the
    PSUM->SBUF row copies overlap remaining compute.

    Each entry: dict(x=tile idx, base=psum row, ncols=lhsT cols, start, stop,
    tpos=tile_position, rows=[(plo, phi, co_idx, rlo, rhi, wt_col)],
    const=[(partition, co_idx, wt_col)], fin=[channels finalized])."""
    def full(c, j):
        return dict(x=c, base=32 * j, ncols=1, start=(c == 0), stop=(c == 2),
                    tpos=(0, 32 * j), rows=[(0, 128, j, c * 128, (c + 1) * 128, 0)],
                    const=[], fin=([j] if c == 2 else []))

    def tailp(jA, jB):
        # const rows: hi/lo split so a bf16 weight tensor still carries the
        # channel constant to ~fp32 accuracy (two saturated-tanh ones-rows)
        return dict(x=3, base=32 * jA, ncols=64, start=False, stop=False,
                    tpos=(0, 32 * jA),
                    rows=[(0, TAIL, jA, NFULL * 128, R, 0),
                          (TAIL, 2 * TAIL, jB, NFULL * 128, R, 32)],
                    const=[(96, jA, 0, "hi"), (98, jA, 0, "lo"),
                           (97, jB, 32, "hi"), (99, jB, 32, "lo")],
                    fin=[])

    # per channel the accumulation order is c0 (start), tail, c1, c2
    # (stop+fin), so every channel finalizes on a full-chunk iteration and
    # the tail iterations sit mid-stream.
    specs = []
    specs.append(full(0, 0))
    specs.append(full(0, 1))
    specs.append(tailp(0, 1))
    specs.append(full(1, 0))
    specs.append(full(1, 1))
    specs.append(full(2, 0))
    specs.append(full(2, 1))
    specs.append(full(0, 2))
    specs.append(full(0, 3))
    specs.append(tailp(2, 3))
    specs.append(full(1, 2))
    specs.append(full(1, 3))
    specs.append(full(2, 2))
    specs.append(full(2, 3))
    return specs


def _build_bass(mm_dtype=mybir.dt.float32, reps=1):
    nc = bacc.Bacc(
        "TRN2",
        target_bir_lowering=False,
        debug=False,
        enable_asserts=False,
        num_devices=NCORES,
    )
    f32 = mybir.dt.float32
    xx = nc.dram_tensor("xx", [4, 128, F], f32, kind="ExternalInput")
    par = nc.dram_tensor("par", [128, NITER, 4], f32, kind="ExternalInput")
    wt = nc.dram_tensor("wt", [128, NITER, 64], mm_dtype, kind="ExternalInput")
    out = nc.dram_tensor("out", [4, F], f32, kind="ExternalOutput")

    with ExitStack() as ctx:
        tc = ctx.enter_context(tile.TileContext(nc))
        singles = ctx.enter_context(tc.tile_pool(name="singles", bufs=1))
        xpool = ctx.enter_context(tc.tile_pool(name="xpool", bufs=1))
        small_v = mm_dtype in (mybir.dt.bfloat16, mybir.dt.float16)
        vb = 3 if small_v else 2
        tb = 3 if small_v else 2
        spool = ctx.enter_context(tc.tile_pool(name="spool", bufs=3))
        tpool = ctx.enter_context(tc.tile_pool(name="tpool", bufs=tb))
        vpool = ctx.enter_context(tc.tile_pool(name="vpool", bufs=vb))
        psum_pool = ctx.enter_context(tc.tile_pool(name="psum", bufs=1, space="PSUM"))

        # Warm the activation table set (sigmoid_and_others, includes tanh)
        # before any DMA completes, so the ~2.7us load is off the critical path.
        zt = singles.tile([1, 1], f32, tag="zt")
        nc.vector.memset(zt[:], 0.0)
        nc.scalar.activation(zt[:], zt[:], mybir.ActivationFunctionType.Sigmoid)

        # DMA order follows first-use: params, x0 (in quarters so the first
        # iteration can start on the first quarter), tail tile x3, x1, wt, x2.
        par_sb = singles.tile([128, NITER, 4], f32, tag="par")
        nc.sync.dma_start(par_sb[:], par[:, :, :])
        xts = []
        for i in range(4):
            xt = xpool.tile([128, F], f32, tag=f"x{i}")
            xts.append(xt)
        for q in range(4):
            nc.sync.dma_start(xts[0][:, q * 1024:(q + 1) * 1024],
                              xx[0, :, q * 1024:(q + 1) * 1024])
        nc.sync.dma_start(xts[3][:], xx[3, :, :])
        nc.sync.dma_start(xts[1][:], xx[1, :, :])
        wt_sb = singles.tile([128, NITER, 64], mm_dtype, tag="wt")
        nc.sync.dma_start(wt_sb[:], wt[:, :, :])
        nc.sync.dma_start(xts[2][:], xx[2, :, :])

        psum_t = psum_pool.tile([128, F], f32, tag="acc")
        out_sb = singles.tile([128, F], f32, tag="osb")

        Act = mybir.ActivationFunctionType
        Op = mybir.AluOpType
        specs = _iter_specs()
        nspec = len(specs)
        for rep in range(reps):
          for i, sp in enumerate(specs):
            xt = xts[sp["x"]]
            # first iteration in quarters (overlaps the piecewise x0 DMA),
            # last two iterations in halves (pipelines the kernel drain)
            npiece = 4 if i == 0 else (4 if i == nspec - 1 else 1)
            fp = F // npiece
            for q in range(npiece):
                flo, fhi = q * fp, (q + 1) * fp
                s_t = spool.tile([128, fp], f32, tag="s")
                nc.scalar.activation(s_t[:], xt[:, flo:fhi], Act.Sigmoid,
                                     bias=par_sb[:, i, 0:1], scale=GATE)
                # t = s*(0.2*Ec) + x; the k multiply and the 0.8*k*Ec add are
                # folded into the tanh activation's per-partition scale/bias
                t_t = tpool.tile([128, fp], f32, tag="t")
                nc.vector.scalar_tensor_tensor(t_t[:], s_t[:],
                                               par_sb[:, i, 3:4],
                                               xt[:, flo:fhi],
                                               Op.mult, Op.add)
                v_t = vpool.tile([128, fp], mm_dtype, tag="v")
                nc.scalar.activation(v_t[:], t_t[:], Act.Tanh,
                                     bias=par_sb[:, i, 2:3],
                                     scale=par_sb[:, i, 1:2])
                nb, nco = sp["base"], sp["ncols"]
                for seg in range(fp // MM_SEG):
                    nc.tensor.matmul(
                        psum_t[nb:nb + nco,
                               flo + seg * MM_SEG:flo + (seg + 1) * MM_SEG],
                        wt_sb[:, i, 0:nco],
                        v_t[:, seg * MM_SEG:(seg + 1) * MM_SEG],
                        start=sp["start"], stop=sp["stop"],
                        tile_position=sp["tpos"],
                    )
                for j in sp["fin"]:
                    src = psum_t[32 * j:32 * j + 1, flo:fhi]
                    dst = out_sb[32 * j:32 * j + 1, flo:fhi]
                    if i == nspec - 1:
                        nc.scalar.copy(dst, src)  # overlaps the PE drain
                    else:
                        nc.vector.tensor_copy(dst, src)
                    nc.sync.dma_start(out[j:j + 1, flo:fhi], dst)

    nc.compile()
    return nc


def _host_prep(x, k, Ec, Ps, bias, coef, out_bias, w_np_dtype):
    """Build the unfolded X tiles (core-independent) and per-core params."""
    f32 = np.float32
    x = np.asarray(x, f32)
    xp = np.pad(x, ((0, 0), (0, 0), (1, 1), (1, 1)))
    # X[r, f]: r = (ci, nb, kh, kw), f = (b, ho, wo)
    Xf = np.empty((Cin, NB, KH, KW, F), f32)
    for kh in range(KH):
        for kw in range(KW):
            win = xp[:, :, kh:kh + H, kw:kw + W]              # [B, Cin, 32, 32]
            win = win.transpose(1, 0, 2, 3).reshape(Cin, F)   # [Cin, F]
            Xf[:, :, kh, kw, :] = win[:, None, :]
    X432 = Xf.reshape(R, F)

    xx = np.zeros((4, 128, F), f32)
    xx[0:NFULL] = X432[0:NFULL * 128].reshape(NFULL, 128, F)
    xx[3, 0:TAIL] = X432[NFULL * 128:]
    xx[3, TAIL:2 * TAIL] = X432[NFULL * 128:]

    k2 = np.asarray(k, f32).reshape(Cout, R)
    Ec2 = np.asarray(Ec, f32).reshape(Cout, R)
    Ps2 = np.asarray(Ps, f32).reshape(Cout, R)
    bias2 = np.asarray(bias, f32).reshape(Cout, R)
    coef2 = np.asarray(coef, f32).reshape(Cout, R)
    ob = np.asarray(out_bias, f32).reshape(Cout)

    b10 = GATE * Ec2
    c1 = ALPHA * k2 * Ec2          # tanh bias
    c2k = (1.0 - ALPHA) * Ec2      # STT scalar (k folded into tanh scale)
    w = coef2 * Ps2
    const = (coef2 * bias2).sum(axis=1) + ob

    specs = _iter_specs()
    in_maps = []
    for d in range(NCORES):
        cos = [d * CO_PER_CORE + jj for jj in range(CO_PER_CORE)]
        PAR = np.zeros((128, NITER, 4), f32)
        WT = np.zeros((128, NITER, 64), f32)
        for i, sp in enumerate(specs):
            for (plo, phi, j, rlo, rhi, col) in sp["rows"]:
                co = cos[j]
                PAR[plo:phi, i, 0] = b10[co, rlo:rhi]
                PAR[plo:phi, i, 1] = k2[co, rlo:rhi]
                PAR[plo:phi, i, 2] = c1[co, rlo:rhi]
                PAR[plo:phi, i, 3] = c2k[co, rlo:rhi]
                WT[plo:phi, i, col] = w[co, rlo:rhi]
            for (p, j, col, part) in sp["const"]:
                # arg = 25 -> tanh = 1.0 exactly; weight = channel constant
                PAR[p, i, 2] = 25.0
                hi = w_np_dtype(np.float32(const[cos[j]]))
                if part == "hi":
                    WT[p, i, col] = np.float32(hi)
                else:
                    WT[p, i, col] = np.float32(const[cos[j]]) - np.float32(hi)
        in_maps.append({
            "xx": xx,
            "par": PAR,
            "wt": WT.astype(w_np_dtype),
        })
    return in_maps


_nc_cache = {}
last_results = None  # BassKernelResults from the most recent run

_MM_MODES = {
    "fp32": (mybir.dt.float32, np.float32),
    "fp16": (mybir.dt.float16, np.float16),
    "bf16": (mybir.dt.bfloat16, ml_dtypes.bfloat16),
}
MM_MODE = "fp16"


def _get_nc():
    key = MM_MODE
    if key not in _nc_cache:
        _nc_cache[key] = _build_bass(mm_dtype=_MM_MODES[key][0])
    return _nc_cache[key]


def kernel(x, k, Ec, Ps, bias, coef, out_bias, _trace=False):
    global last_results
    in_maps = _host_prep(x, k, Ec, Ps, bias, coef, out_bias, _MM_MODES[MM_MODE][1])
    try:
        res = run_bass_kernel_spmd(_get_nc(), in_maps,
                                   core_ids=list(range(NCORES)), trace=_trace)
    except ModuleNotFoundError:
        # axon NTFF profiling hook unavailable -> run without trace
        res = run_bass_kernel_spmd(_get_nc(), in_maps,
                                   core_ids=list(range(NCORES)), trace=False)
    last_results = res
    o = np.concatenate([r["out"] for r in res.results], axis=0)  # [32, F]
    o = o.reshape(Cout, B, H, W).transpose(1, 0, 2, 3)
    return np.ascontiguousarray(o.astype(np.float32))
